# revision 11
# baseline (speedup 1.0000x reference)
"""BlockStackingSGN kernel for 8 Trainium2 NeuronCores.

Strategy: data-parallel over batch B=4096 -> 512 rows per core; all MLP
weights replicated. On-chip layout keeps activations transposed
([hidden -> partitions, batch -> free]) so every matmul streams the batch
through the PE with the weight stationary (bf16 operands, fp32 PSUM
accumulation). The three 256->1 output heads (clear / ontable / AonB) are
folded into one PSUM accumulation bank: each head's weight column is
embedded at output-row position r of a [128,128] stationary operand, so
all 80 output rows accumulate into a single [128, 512] bank and one
batched Sigmoid finishes the kernel. Elementwise work is spread across
the Scalar, Vector, and GpSimd engines to keep them all under the PE's
span; all 128-partition weights travel in one packed SBUF tile DMA'd in
parallel chunks.
"""

import sys

import numpy as np

sys.path.insert(0, "/opt/trn_rl_repo")

import concourse.bacc as bacc
import concourse.mybir as mybir
import concourse.tile as tile
from concourse.bass_utils import run_bass_kernel_spmd

dt = mybir.dt
AF = mybir.ActivationFunctionType
ALU = mybir.AluOpType

N = 8          # blocks
H = 256        # hidden
B = 4096       # batch
IN = 3 * N     # 24
NCORES = 8
BC = B // NCORES   # 512 batch rows per core
KT = H // 128      # k-tiles per 256-wide contraction
R = N * (N + 2)    # 80 output rows per batch element

F32 = dt.float32
BF16 = dt.bfloat16
W = BC

_CACHE = {}


def _wb_layout():
    """Column layout of the packed [128, ncols] bf16 weight tile.
    Order doubles as DMA arrival order: block-0 weights first."""
    keys = []
    for n in range(N):
        for k in range(KT):
            keys.append(("oW1", n, k))
            keys.append(("oW2", n, k))
    for nm in ("cW0", "cW1", "tW0", "tW1"):
        for k in range(KT):
            keys.append((nm, k))
    for nm in ("aW0l", "aW0r", "aW1"):
        for k in range(KT):
            keys.append((nm, k))
    for t_ in range(3):
        for k in range(KT):
            for p in range(2):
                keys.append(("w2e", t_, k, p))
    off = {}
    col = 0
    for key in keys:
        off[key] = col
        col += H
    return off, col


WB_OFF, WB_COLS = _wb_layout()

# bias tile column layout ([128, nb] fp32)
def _bias_layout():
    keys = []
    for n in range(N):
        for nm in ("ob0", "ob1", "ob2"):
            for m in range(KT):
                keys.append((nm, n, m))
    for nm in ("cb0", "cb1", "tb0", "tb1", "ab0", "ab1"):
        for m in range(KT):
            keys.append((nm, m))
    keys.append(("finb",))
    return {k: i for i, k in enumerate(keys)}, len(keys)


BIAS_OFF, BIAS_COLS = _bias_layout()

N_DMA_CHUNKS = 8


def _build():
    nc = bacc.Bacc("TRN2", target_bir_lowering=False, debug=False, num_devices=NCORES)

    d_xT = nc.dram_tensor("xT", [IN, BC], BF16, kind="ExternalInput")
    d_wb = nc.dram_tensor("wb", [128, WB_COLS], BF16, kind="ExternalInput")
    d_ow0 = nc.dram_tensor("ow0", [IN, N * H], BF16, kind="ExternalInput")
    d_bias = nc.dram_tensor("bias", [128, BIAS_COLS], F32, kind="ExternalInput")
    d_out = nc.dram_tensor("outT", [R, BC], F32, kind="ExternalOutput")

    with tile.TileContext(nc) as tc:
        with (
            tc.tile_pool(name="w", bufs=1) as wp,
            tc.tile_pool(name="act", bufs=1) as ap,
            tc.tile_pool(name="wk", bufs=6) as wk,
            tc.tile_pool(name="ps", bufs=7, space="PSUM") as ps,
            tc.tile_pool(name="finp", bufs=1, space="PSUM") as fp,
        ):
            xT = wp.tile([IN, BC], BF16, tag="xT")
            nc.sync.dma_start(xT[:], d_xT[:])
            ow0 = wp.tile([IN, N * H], BF16, tag="ow0")
            nc.gpsimd.dma_start(ow0[:], d_ow0[:])
            bias = wp.tile([128, BIAS_COLS], F32, tag="bias")
            nc.sync.dma_start(bias[:], d_bias[:])

            wb = wp.tile([128, WB_COLS], BF16, tag="wb")
            chunk = (WB_COLS + N_DMA_CHUNKS - 1) // N_DMA_CHUNKS
            for c in range(N_DMA_CHUNKS):
                eng = nc.gpsimd if c % 2 == 0 else nc.sync
                sl = slice(c * chunk, min((c + 1) * chunk, WB_COLS))
                eng.dma_start(wb[:, sl], d_wb[:, sl])

            def wslice(*key):
                o = WB_OFF[key]
                return lambda m: wb[:, o + m * 128 : o + (m + 1) * 128]

            def bcol(*key):
                i = BIAS_OFF[key]
                return bias[:, i : i + 1]

            fin = fp.tile([128, BC], F32, tag="fin")
            n_fin = 2 * KT * N + KT * N * N
            fin_ct = [0]

            def fin_mm(t_, k, r, rhs):
                first = fin_ct[0] == 0
                fin_ct[0] += 1
                last = fin_ct[0] == n_fin
                p = r % 2
                o = WB_OFF[("w2e", t_, k, p)]
                lhsT = wb[:, o + 128 + p - r : o + 256 + p - r]
                nc.tensor.matmul(fin[:], lhsT, rhs, start=first, stop=last)

            # weighted round-robin of PSUM evacuations; ratio set per phase
            evac_ct = [0]
            ev_pat = ["AD"]

            def evac(out_ap, psum_ap, bias_ap, relu):
                e = ev_pat[0][evac_ct[0] % len(ev_pat[0])]
                evac_ct[0] += 1
                if e == "A":
                    func = AF.Relu if relu else (AF.Identity if bias_ap is not None else AF.Copy)
                    if bias_ap is not None:
                        nc.scalar.activation(out_ap, psum_ap, func, bias=bias_ap)
                    else:
                        nc.scalar.activation(out_ap, psum_ap, func)
                else:
                    if relu:
                        b = bias_ap if bias_ap is not None else 0.0
                        nc.vector.tensor_scalar(out_ap, psum_ap, b, 0.0, ALU.add, ALU.max)
                    elif bias_ap is not None:
                        nc.vector.tensor_scalar(out_ap, psum_ap, bias_ap, None, ALU.add)
                    else:
                        nc.vector.tensor_copy(out_ap, psum_ap)

            def layer(wgt, bias_m, in_tile, in_parts, relu, out_pool, out_tag):
                """One 256-out layer -> [128, KT*W] tile.
                wgt(k) -> fn m -> lhsT AP; in_parts: list of rhs APs (k-tiles)."""
                out = out_pool.tile([128, KT * W], BF16, tag=out_tag)
                if in_parts is None:
                    in_parts = [in_tile[:, k * W : (k + 1) * W] for k in range(KT)]
                for m in range(KT):
                    pst = ps.tile([128, BC], F32, tag="ps")
                    for ki, rhs in enumerate(in_parts):
                        nc.tensor.matmul(pst[:], wgt(ki)(m), rhs,
                                         start=(ki == 0), stop=(ki == len(in_parts) - 1))
                    evac(out[:, m * W : (m + 1) * W], pst[:],
                         bias_m(m) if bias_m else None, relu)
                return out

            # ---- object encoders -> enc[n] [128, 2W] (persistent) ----
            enc = []
            for n in range(N):
                ow0_l = lambda n=n: (lambda ki: (lambda m: ow0[:, n * H + m * 128 : n * H + (m + 1) * 128]))
                h0 = layer(ow0_l(), lambda m, n=n: bcol("ob0", n, m), None, [xT[:]], True, wk, "h")
                h1 = layer(lambda ki, n=n: wslice("oW1", n, ki), lambda m, n=n: bcol("ob1", n, m),
                           h0, None, True, wk, "h")
                e = layer(lambda ki, n=n: wslice("oW2", n, ki), lambda m, n=n: bcol("ob2", n, m),
                          h1, None, False, ap, f"enc_{n}")
                enc.append(e)

            # ---- AonB pair-input halves (bias a_b0 folded into left) ----
            al, ar = [], []
            for n in range(N):
                al.append(layer(lambda ki: wslice("aW0l", ki), lambda m: bcol("ab0", m),
                                enc[n], None, False, ap, f"al_{n}"))
                ar.append(layer(lambda ki: wslice("aW0r", ki), None,
                                enc[n], None, False, ap, f"ar_{n}"))

            # ---- clear / ontable predicate thunks (interleaved into pairs) ----
            def pred_thunk(n, w0nm, w1nm, b0nm, b1nm, w2idx, r):
                def go():
                    y0 = layer(lambda ki: wslice(w0nm, ki),
                               lambda m: bcol(b0nm, m), enc[n], None, True, wk, "h")
                    y1 = layer(lambda ki: wslice(w1nm, ki),
                               lambda m: bcol(b1nm, m), y0, None, True, wk, "h")
                    for k in range(KT):
                        fin_mm(w2idx, k, r, y1[:, k * W : (k + 1) * W])
                return go

            preds = []
            for n in range(N):
                preds.append(pred_thunk(n, "cW0", "cW1", "cb0", "cb1", 0, n * 10 + 8))
                preds.append(pred_thunk(n, "tW0", "tW1", "tb0", "tb1", 1, n * 10 + 9))

            # ---- all (i, j) pairs, ordered round-robin across the three
            # 32-row PSUM partition groups ----
            zero = wp.tile([128, KT * W], BF16, tag="zero")
            nc.gpsimd.memset(zero[:], 0.0)
            ev_pat[0] = "AAAD"  # pair phase: DVE busy with add/relu chains
            buckets = [[], [], []]
            for i in range(N):
                for j in range(N):
                    buckets[(i * 10 + j) // 32].append((i, j))
            order = []
            bi = 0
            while any(buckets):
                if buckets[bi % 3]:
                    order.append(buckets[bi % 3].pop(0))
                bi += 1
            for pi, (i, j) in enumerate(order):
                if pi % 4 == 0 and preds:
                    preds.pop(0)()
                r = i * 10 + j
                phs = wk.tile([128, KT * W], BF16, tag="phs")
                nc.vector.tensor_tensor(phs[:], al[i][:], ar[j][:], ALU.add)
                ph = wk.tile([128, KT * W], BF16, tag="ph")
                nc.vector.tensor_tensor(ph[:], phs[:], zero[:], ALU.max)
                y = layer(lambda ki: wslice("aW1", ki), lambda m: bcol("ab1", m),
                          ph, None, True, wk, "y")
                for k in range(KT):
                    fin_mm(2, k, r, y[:, k * W : (k + 1) * W])
            for t in preds:
                t()

            assert fin_ct[0] == n_fin

            # ---- batched sigmoid over all 80 head rows + store ----
            outT = wk.tile([128, BC], F32, tag="outT")
            nc.scalar.activation(outT[:], fin[:], AF.Sigmoid, bias=bcol("finb"))
            nc.sync.dma_start(d_out[:], outT[:R, :])

    nc.compile()
    return nc


def _prep_inputs(inputs):
    import ml_dtypes

    bf = ml_dtypes.bfloat16
    f32a = lambda a: np.asarray(a, dtype=np.float32)

    wbv = np.zeros((128, WB_COLS), bf)

    def put(key, arr):  # arr: [128, 256] fp32
        o = WB_OFF[key]
        wbv[:, o : o + H] = arr.astype(bf)

    oW1 = f32a(inputs["o_W1"])
    oW2 = f32a(inputs["o_W2"])
    for n in range(N):
        for k in range(KT):
            put(("oW1", n, k), oW1[n, k * 128 : (k + 1) * 128])
            put(("oW2", n, k), oW2[n, k * 128 : (k + 1) * 128])
    for nm, src in (("cW0", "c_W0"), ("cW1", "c_W1"), ("tW0", "t_W0"), ("tW1", "t_W1")):
        a = f32a(inputs[src])
        for k in range(KT):
            put((nm, k), a[k * 128 : (k + 1) * 128])
    aW0 = f32a(inputs["a_W0"])
    for k in range(KT):
        put(("aW0l", k), aW0[k * 128 : (k + 1) * 128])
        put(("aW0r", k), aW0[H + k * 128 : H + (k + 1) * 128])
    aW1 = f32a(inputs["a_W1"])
    for k in range(KT):
        put(("aW1", k), aW1[k * 128 : (k + 1) * 128])
    for t_, src2 in enumerate(("c_W2", "t_W2", "a_W2")):
        w2 = f32a(inputs[src2])[:, 0].astype(bf)
        for k in range(KT):
            for p in range(2):
                o = WB_OFF[("w2e", t_, k, p)]
                wbv[:, o + 128 + p] = w2[k * 128 : (k + 1) * 128]

    biasv = np.zeros((128, BIAS_COLS), np.float32)

    def putb(key, vec128):
        biasv[:, BIAS_OFF[key]] = vec128

    for n in range(N):
        for nm, src in (("ob0", "o_b0"), ("ob1", "o_b1"), ("ob2", "o_b2")):
            a = f32a(inputs[src])[n]
            for m in range(KT):
                putb((nm, n, m), a[m * 128 : (m + 1) * 128])
    for nm, src in (("cb0", "c_b0"), ("cb1", "c_b1"), ("tb0", "t_b0"),
                    ("tb1", "t_b1"), ("ab0", "a_b0"), ("ab1", "a_b1")):
        a = f32a(inputs[src])
        for m in range(KT):
            putb((nm, m), a[m * 128 : (m + 1) * 128])
    finb = np.zeros(128, np.float32)
    for i in range(N):
        finb[i * 10 : i * 10 + 8] = f32a(inputs["a_b2"])[0]
        finb[i * 10 + 8] = f32a(inputs["c_b2"])[0]
        finb[i * 10 + 9] = f32a(inputs["t_b2"])[0]
    putb(("finb",), finb)

    ow0v = np.zeros((IN, N * H), bf)
    oW0 = f32a(inputs["o_W0"])
    for n in range(N):
        ow0v[:, n * H : (n + 1) * H] = oW0[n].astype(bf)

    xT = np.ascontiguousarray(f32a(inputs["x"]).T)  # (24, 4096)
    common = {"wb": wbv, "ow0": ow0v, "bias": biasv}
    in_maps = []
    for c in range(NCORES):
        m = dict(common)
        m["xT"] = np.ascontiguousarray(xT[:, c * BC : (c + 1) * BC]).astype(bf)
        in_maps.append(m)
    return in_maps


def run(inputs, trace=False, **kw):
    if "nc" not in _CACHE:
        _CACHE["nc"] = _build()
    nc = _CACHE["nc"]
    in_maps = _prep_inputs(inputs)
    res = run_bass_kernel_spmd(nc, in_maps, list(range(NCORES)), trace=trace, **kw)
    out = np.concatenate([res.results[c]["outT"].T for c in range(NCORES)], axis=0)
    return out.astype(np.float32), res


def kernel(**inputs) -> np.ndarray:
    out, _ = run(inputs, trace=False)
    return out
